# revision 1
# baseline (speedup 1.0000x reference)
"""Contrastive-loss Trainium2 kernel: symmetry-halved fp8 DoubleRow GEMM + AllGather.

zn is scaled by 16 before the fp8 cast (values ~0.5 fit e4m3 comfortably);
the GEMM result is 256*cos, compensated in the exp scale and pos scale.

cos_sim is symmetric, so only block-distances d ∈ {0..4} are computed per
core (columns local [0:5120) in the rotated frame); the exp-sums for
distances 5,6,7 of each row are the COLUMN sums of the d ∈ {3,2,1} blocks
computed by cores c+5, c+6, c+7. Each core:
  - computes its 1024 x 5120 block of exp(S/T) with diag masked,
  - row-sums it (ACT fused accum),
  - column-sums the d in {1,2,3} sub-blocks (elementwise accumulate over
    the 8 row-tiles on DVE, then a partition-reduce via a ones-matmul),
  - AllGathers packet = [rowsums(1024), cs_d1(1024), cs_d2(1024),
    cs_d3(1024)] (fp32, 16KB/rank),
  - reassembles the GLOBAL per-row totals (the gathered frame is
    rank-indexed, so placement is static), takes ln, and reduces
    Sum_r ln(total_r) over all 8192 rows (identical on every core).
Output per core: [128,1] partial = (Sum ln)/8 - Sum_own(pos)/T partials.
Host sums 8x128 values / 8192.
"""

import os
from contextlib import ExitStack

import numpy as np

N = 8192
D = 1024
N_CORES = 8
ROWS_PER_CORE = N // N_CORES  # 1024
P = 128
TEMPERATURE = 0.07
INV_T = 1.0 / TEMPERATURE
MASK_VAL = -65504.0
SCALE = 16.0  # pre-fp8 scale; psum holds SCALE^2 * cos

NBLK = 5  # block distances 0..4 computed locally
COLS = NBLK * ROWS_PER_CORE  # 5120 local columns
N_ROW_TILES = COLS // P  # 40 row tiles to normalize (rows [0:5120))
MB = ROWS_PER_CORE // P  # 8
KT = D // P  # 8
COLG = 512
NB = COLS // COLG  # 10 column tiles
CS_NB = range(2, 8)  # col tiles covering d in {1,2,3} (cols 1024:4096)
PKT = 4 * ROWS_PER_CORE  # packet floats: rowsum + 3 colsum blocks

_CACHE = {}


def _build_nc(repeat=1):
    import concourse.mybir as mybir
    import concourse.tile as tile
    from concourse import bacc
    from concourse.masks import make_identity

    f32 = mybir.dt.float32
    bf16 = mybir.dt.bfloat16
    fp8 = mybir.dt.float8e4
    AF = mybir.ActivationFunctionType
    ALU = mybir.AluOpType

    nc = bacc.Bacc("TRN2")
    z_in = nc.dram_tensor("z", [N, D], f32, kind="ExternalInput")
    out_dram = nc.dram_tensor("out", [P, 1], f32, kind="ExternalOutput")
    pkt_dram = nc.dram_tensor("pkt", [PKT], f32)
    gathered = nc.dram_tensor("gathered", [N_CORES, PKT], f32, addr_space="Shared")

    ctx = ExitStack()
    with ctx:
        tc = ctx.enter_context(tile.TileContext(nc))
        consts = ctx.enter_context(tc.tile_pool(name="consts", bufs=1))
        znt_pool = ctx.enter_context(tc.tile_pool(name="znt", bufs=1))
        work = ctx.enter_context(tc.tile_pool(name="work", bufs=3))
        zin = ctx.enter_context(tc.tile_pool(name="zin", bufs=8))
        small = ctx.enter_context(tc.tile_pool(name="small", bufs=4))
        accp = ctx.enter_context(tc.tile_pool(name="accp", bufs=1))
        colp = ctx.enter_context(tc.tile_pool(name="colp", bufs=1))
        psum_t = ctx.enter_context(tc.tile_pool(name="psum_t", bufs=2, space="PSUM"))
        psum_mm = ctx.enter_context(tc.tile_pool(name="psum_mm", bufs=4, space="PSUM"))
        psum_cs = ctx.enter_context(tc.tile_pool(name="psum_cs", bufs=2, space="PSUM"))

        ident_f32 = consts.tile([P, P], f32, tag="ident_f32")
        make_identity(nc, ident_f32)
        ident_bf16 = consts.tile([P, P], bf16, tag="ident_bf16")
        make_identity(nc, ident_bf16)
        ident_fp8 = consts.tile([P, P], fp8, tag="ident_fp8")
        make_identity(nc, ident_fp8)
        negtile = consts.tile([P, P], f32, tag="negtile")
        nc.vector.memset(negtile, MASK_VAL * SCALE * SCALE)
        ident_u8 = consts.tile([P, P], mybir.dt.uint8, tag="ident_u8")
        nc.vector.tensor_copy(ident_u8, ident_f32)
        ones_col = consts.tile([P, 1], bf16, tag="ones_col")
        nc.vector.memset(ones_col, 1.0)

        znt = [
            znt_pool.tile([P, KT, COLG], fp8, tag=f"znt{g}", name=f"znt{g}")
            for g in range(NB)
        ]

        accs = accp.tile([P, MB, NB], f32, tag="accs")
        posq = accp.tile([P, MB], f32, tag="posq")
        # colT[j] accumulates sum over the 8 row-tiles of exp'd tile nb=2+j
        colT = [
            colp.tile([P, COLG], f32, tag=f"colT{j}", name=f"colT{j}")
            for j in range(len(CS_NB))
        ]
        cs = colp.tile([P, 24], f32, tag="cs")  # colsums, col c = local col chunk

        for _rep in range(repeat):
            # ---- phase 1: normalize + transpose (rows [0:5120) only) ----
            for t in range(N_ROW_TILES):
                zt = zin.tile([P, 2, D // 2], f32, tag="zt")
                nc.sync.dma_start(
                    out=zt,
                    in_=z_in[t * P : (t + 1) * P, :].rearrange(
                        "p (a b) -> p a b", a=2
                    ),
                )
                stats = small.tile([P, 2, 6], f32, tag="stats")
                nc.vector.bn_stats(out=stats[:, 0, :], in_=zt[:, 0, :])
                nc.vector.bn_stats(out=stats[:, 1, :], in_=zt[:, 1, :])
                mv = small.tile([P, 2], f32, tag="mv")
                nc.vector.bn_aggr(out=mv, in_=stats)
                m2 = small.tile([P, 1], f32, tag="m2")
                nc.vector.tensor_mul(m2, mv[:, 0:1], mv[:, 0:1])
                s2 = small.tile([P, 1], f32, tag="s2")
                nc.vector.tensor_add(s2, m2, mv[:, 1:2])
                nrm = small.tile([P, 1], f32, tag="nrm")
                nc.scalar.activation(nrm, s2, AF.Sqrt, scale=float(D) / (SCALE * SCALE))
                rinv = small.tile([P, 1], f32, tag="rinv")
                nc.vector.reciprocal(rinv, nrm)

                zn_row = work.tile([P, D], bf16, tag="zn_row")
                nc.vector.tensor_scalar_mul(
                    zn_row.rearrange("p (a b) -> p a b", a=2), zt, rinv
                )

                ptr = psum_t.tile([P, KT * P], bf16, tag="ptr")
                for kk in range(KT):
                    nc.tensor.transpose(
                        ptr[:, kk * P : (kk + 1) * P],
                        zn_row[:, kk * P : (kk + 1) * P],
                        ident_bf16,
                    )
                g, col = t // 4, (t % 4) * P
                dst = znt[g][:, :, col : col + P]
                src = ptr.rearrange("p (k c) -> p k c", k=KT)
                if t % 2 == 0:
                    nc.scalar.copy(dst, src)
                else:
                    nc.vector.tensor_copy(dst, src)

            # zero colsum accumulators
            for j in range(len(CS_NB)):
                nc.vector.memset(colT[j], 0.0)

            # ---- phase 2: GEMM + exp row-sums + colsum accumulation ----
            for nb in range(NB):
                for mb in range(MB):
                    ps = psum_mm.tile([P, COLG], f32, tag="ps")
                    lg, lcol = mb // 4, (mb % 4) * P
                    for kk in range(0, KT, 2):
                        nc.tensor.matmul(
                            ps,
                            lhsT=znt[lg][:, kk : kk + 2, lcol : lcol + P],
                            rhs=znt[nb][:, kk : kk + 2, :],
                            perf_mode=mybir.MatmulPerfMode.DoubleRow,
                            start=(kk == 0),
                            stop=(kk == KT - 2),
                        )
                    if nb == mb // 4:
                        off = (mb % 4) * P
                        nc.vector.copy_predicated(
                            out=ps[:, off : off + P], mask=ident_u8, data=negtile
                        )
                    if nb == 8 + mb // 4:
                        off = (mb % 4) * P
                        pos_scr = work.tile([P, P], f32, tag="pos_scr")
                        nc.vector.tensor_mul(pos_scr, ps[:, off : off + P], ident_f32)
                        nc.vector.tensor_reduce(
                            posq[:, mb : mb + 1],
                            pos_scr,
                            axis=mybir.AxisListType.X,
                            op=ALU.add,
                        )
                    ex = work.tile([P, COLG], bf16, tag="ex")
                    nc.scalar.activation(
                        ex, ps, AF.Exp, scale=INV_T / (SCALE * SCALE),
                        accum_out=accs[:, mb, nb : nb + 1],
                    )
                    if nb in CS_NB:
                        j = nb - 2
                        nc.vector.tensor_add(colT[j], colT[j], ex)

            # ---- colsum partition-reduce via ones-matmul ----
            for j in range(len(CS_NB)):
                ctb = work.tile([P, COLG], bf16, tag="ctb")
                nc.vector.tensor_copy(ctb, colT[j])
                for q in range(COLG // P):
                    cps = psum_cs.tile([P, 1], f32, tag="cps")
                    nc.tensor.matmul(
                        cps,
                        lhsT=ctb[:, q * P : (q + 1) * P],
                        rhs=ones_col,
                        start=True,
                        stop=True,
                    )
                    nc.scalar.copy(cs[:, 4 * j + q : 4 * j + q + 1], cps)

            # ---- pack + AllGather ----
            rowsum = accp.tile([P, MB], f32, tag="rowsum")
            nc.vector.tensor_reduce(
                rowsum, accs, axis=mybir.AxisListType.X, op=ALU.add
            )
            d1 = nc.sync.dma_start(
                out=pkt_dram[0:ROWS_PER_CORE].rearrange("(m p) -> p m", p=P),
                in_=rowsum,
            )
            d2 = nc.sync.dma_start(
                out=pkt_dram[ROWS_PER_CORE:PKT].rearrange("(c p) -> p c", p=P),
                in_=cs,
            )
            cc = nc.gpsimd.collective_compute(
                "AllGather",
                mybir.AluOpType.bypass,
                ins=[pkt_dram.ap()],
                outs=[gathered.ap()],
                replica_groups=[list(range(N_CORES))],
            )
            from concourse.bass import _add_dep_helper

            _add_dep_helper(cc.ins, d1.ins, reason="cc after pkt rowsum")
            _add_dep_helper(cc.ins, d2.ins, reason="cc after pkt cs")

            # ---- reassemble global totals; ln; global reduce ----
            # tot[p, b, m] = total exp-sum for global row 1024 b + 128 m + p
            Rt = accp.tile([P, N_CORES, MB], f32, tag="Rt")
            tot = accp.tile([P, N_CORES, MB], f32, tag="tot")
            Cd = {
                d: accp.tile([P, N_CORES, MB], f32, tag=f"Cd{d}", name=f"Cd{d}")
                for d in (1, 2, 3)
            }
            for b in range(N_CORES):
                dr = nc.sync.dma_start(
                    out=Rt[:, b, :],
                    in_=gathered[b, 0:ROWS_PER_CORE].rearrange("(m p) -> p m", p=P),
                )
                _add_dep_helper(dr.ins, cc.ins, reason="read gathered after cc")
                for d in (1, 2, 3):
                    s = (b - d) % N_CORES
                    dc = nc.sync.dma_start(
                        out=Cd[d][:, b, :],
                        in_=gathered[
                            s, d * ROWS_PER_CORE : (d + 1) * ROWS_PER_CORE
                        ].rearrange("(m p) -> p m", p=P),
                    )
                    _add_dep_helper(dc.ins, cc.ins, reason="read gathered after cc")
            nc.vector.tensor_copy(tot, Rt)
            for d in (1, 2, 3):
                nc.vector.tensor_add(tot, tot, Cd[d])

            lnt = accp.tile([P, N_CORES, MB], f32, tag="lnt")
            nc.scalar.activation(lnt, tot, AF.Ln)
            gsum = accp.tile([P, 1], f32, tag="gsum")
            nc.vector.tensor_reduce(
                gsum, lnt, axis=mybir.AxisListType.XY, op=ALU.add
            )
            poss = accp.tile([P, MB], f32, tag="poss")
            nc.vector.tensor_scalar_mul(poss, posq, -INV_T / (SCALE * SCALE))
            psum_part = accp.tile([P, 1], f32, tag="psum_part")
            nc.vector.tensor_reduce(
                psum_part, poss, axis=mybir.AxisListType.X, op=ALU.add
            )
            part = accp.tile([P, 1], f32, tag="part")
            nc.vector.tensor_scalar_mul(part, gsum, 1.0 / N_CORES)
            nc.vector.tensor_add(part, part, psum_part)
            nc.sync.dma_start(out=out_dram[:, :], in_=part)

    nc.finalize()
    return nc


def _get_nc():
    if "nc" not in _CACHE:
        _CACHE["nc"] = _build_nc()
    return _CACHE["nc"]


def _run(z, trace=False):
    from concourse.bass_utils import run_bass_kernel_spmd

    z = np.ascontiguousarray(np.asarray(z, dtype=np.float32))
    assert z.shape == (N, D), z.shape
    nc = _get_nc()
    in_maps = [
        {"z": np.ascontiguousarray(np.roll(z, -ROWS_PER_CORE * c, axis=0))}
        for c in range(N_CORES)
    ]
    res = run_bass_kernel_spmd(
        nc, in_maps, core_ids=list(range(N_CORES)), trace=False
    )
    total = np.float64(0.0)
    for r in res.results:
        total += r["out"].astype(np.float64).sum()
    loss = np.float32(total / N)
    return loss, res


def kernel(z):
    loss, _ = _run(z, trace=False)
    return np.array(loss, dtype=np.float32)



# revision 2
# speedup vs baseline: 1.5210x; 1.5210x over previous
"""Contrastive-loss TRN2 kernel v3: chunked slab exchange + leaner tail.

Changes over v2:
- The own-rows fp8 slab is AllGathered in 4 column chunks (256 cols each),
  each fired as soon as its 2 phase-A tiles are done, so transport overlaps
  phase A and the own-block GEMM (nb 0/1).
- colT colsum accumulators are bf16 (DVE 2x mode), fed straight to the
  ones-matmul as lhsT (no f32->bf16 convert copy).
- cs partition-reduce packs 4 one-col matmuls into one [P,4] psum tile,
  one ACT copy per d-group instead of 4.
- Packet is p-major (contiguous 128B per partition); the reassembly reads
  ALL ranks' packets in a single DMA into G [P, 8, 32] and does 7 small
  DVE adds on views, replacing 32 strided DMAs.
- Phase-A psum->sbuf slab copies alternate ACT/DVE.
"""

import os
from contextlib import ExitStack

import numpy as np

N = 8192
D = 1024
N_CORES = 8
ROWS_PER_CORE = N // N_CORES  # 1024
P = 128
TEMPERATURE = 0.07
INV_T = 1.0 / TEMPERATURE
MASK_VAL = -65504.0
SCALE = 16.0  # pre-fp8 scale; psum holds SCALE^2 * cos

NBLK = 5  # block distances 0..4 computed locally
COLS = NBLK * ROWS_PER_CORE  # 5120 local columns
MB = ROWS_PER_CORE // P  # 8 row tiles (own rows)
KT = D // P  # 8
COLG = 512
NB = COLS // COLG  # 10 column tiles
CS_NB = range(2, 8)  # col tiles covering d in {1,2,3} (cols 1024:4096)
PKTW = 32  # per-partition packet words: 8 rowsum + 24 cs
PKT = P * PKTW  # 4096 f32
NCH = 4  # slab exchange chunks
CHC = ROWS_PER_CORE // NCH  # 256 cols per chunk
CHUNK = P * KT * CHC  # fp8 elements per chunk (256KB)

_CACHE = {}


def _build_nc(repeat=1):
    import concourse.mybir as mybir
    import concourse.tile as tile
    from concourse import bacc
    from concourse.bass import _add_dep_helper
    from concourse.masks import make_identity

    f32 = mybir.dt.float32
    bf16 = mybir.dt.bfloat16
    fp8 = mybir.dt.float8e4
    AF = mybir.ActivationFunctionType
    ALU = mybir.AluOpType

    nc = bacc.Bacc("TRN2")
    z_in = nc.dram_tensor("z", [N, D], f32, kind="ExternalInput")
    out_dram = nc.dram_tensor("out", [P, 1], f32, kind="ExternalOutput")
    zchunk = [
        nc.dram_tensor(f"zchunk{j}", [CHUNK], fp8) for j in range(NCH)
    ]
    gath_zc = [
        nc.dram_tensor(f"gath_zc{j}", [N_CORES, CHUNK], fp8, addr_space="Shared")
        for j in range(NCH)
    ]
    pkt_dram = nc.dram_tensor("pkt", [PKT], f32)
    gathered = nc.dram_tensor("gathered", [N_CORES, PKT], f32, addr_space="Shared")

    ctx = ExitStack()
    with ctx:
        tc = ctx.enter_context(tile.TileContext(nc))
        consts = ctx.enter_context(tc.tile_pool(name="consts", bufs=1))
        znt_pool = ctx.enter_context(tc.tile_pool(name="znt", bufs=1))
        work = ctx.enter_context(tc.tile_pool(name="work", bufs=3))
        zin = ctx.enter_context(tc.tile_pool(name="zin", bufs=4))
        small = ctx.enter_context(tc.tile_pool(name="small", bufs=4))
        accp = ctx.enter_context(tc.tile_pool(name="accp", bufs=1))
        colp = ctx.enter_context(tc.tile_pool(name="colp", bufs=1))
        psum_t = ctx.enter_context(tc.tile_pool(name="psum_t", bufs=2, space="PSUM"))
        psum_mm = ctx.enter_context(tc.tile_pool(name="psum_mm", bufs=4, space="PSUM"))
        psum_cs = ctx.enter_context(tc.tile_pool(name="psum_cs", bufs=2, space="PSUM"))

        ident_f32 = consts.tile([P, P], f32, tag="ident_f32")
        make_identity(nc, ident_f32)
        ident_bf16 = consts.tile([P, P], bf16, tag="ident_bf16")
        make_identity(nc, ident_bf16)
        negtile = consts.tile([P, P], f32, tag="negtile")
        nc.vector.memset(negtile, MASK_VAL * SCALE * SCALE)
        ident_u8 = consts.tile([P, P], mybir.dt.uint8, tag="ident_u8")
        nc.vector.tensor_copy(ident_u8, ident_f32)
        ones_col = consts.tile([P, 1], bf16, tag="ones_col")
        nc.vector.memset(ones_col, 1.0)

        znt = [
            znt_pool.tile([P, KT, ROWS_PER_CORE], fp8, tag=f"znt{d}", name=f"znt{d}")
            for d in range(NBLK)
        ]

        accs = accp.tile([P, MB, NB], f32, tag="accs")
        posq = accp.tile([P, MB], f32, tag="posq")
        colT = [
            colp.tile([P, COLG], bf16, tag=f"colT{j}", name=f"colT{j}")
            for j in range(len(CS_NB))
        ]
        cs = colp.tile([P, 24], f32, tag="cs")

        pid = nc.sync.partition_id()

        for _rep in range(repeat):
            # ---- phase A: normalize + transpose own rows; fire chunk AGs ----
            ccz = [None] * NCH
            for t in range(MB):
                zt = zin.tile([P, 2, D // 2], f32, tag="zt")
                nc.sync.dma_start(
                    out=zt,
                    in_=z_in[t * P : (t + 1) * P, :].rearrange(
                        "p (a b) -> p a b", a=2
                    ),
                )
                stats = small.tile([P, 2, 6], f32, tag="stats")
                nc.vector.bn_stats(out=stats[:, 0, :], in_=zt[:, 0, :])
                nc.vector.bn_stats(out=stats[:, 1, :], in_=zt[:, 1, :])
                mv = small.tile([P, 2], f32, tag="mv")
                nc.vector.bn_aggr(out=mv, in_=stats)
                m2 = small.tile([P, 1], f32, tag="m2")
                nc.vector.tensor_mul(m2, mv[:, 0:1], mv[:, 0:1])
                s2 = small.tile([P, 1], f32, tag="s2")
                nc.vector.tensor_add(s2, m2, mv[:, 1:2])
                nrm = small.tile([P, 1], f32, tag="nrm")
                nc.scalar.activation(nrm, s2, AF.Sqrt, scale=float(D) / (SCALE * SCALE))
                rinv = small.tile([P, 1], f32, tag="rinv")
                nc.vector.reciprocal(rinv, nrm)

                zn_row = work.tile([P, D], bf16, tag="zn_row")
                nc.vector.tensor_scalar_mul(
                    zn_row.rearrange("p (a b) -> p a b", a=2), zt, rinv
                )

                ptr = psum_t.tile([P, KT * P], bf16, tag="ptr")
                for kk in range(KT):
                    nc.tensor.transpose(
                        ptr[:, kk * P : (kk + 1) * P],
                        zn_row[:, kk * P : (kk + 1) * P],
                        ident_bf16,
                    )
                dst = znt[0][:, :, t * P : (t + 1) * P]
                src = ptr.rearrange("p (k c) -> p k c", k=KT)
                if t % 2 == 0:
                    nc.scalar.copy(dst, src)
                else:
                    nc.vector.tensor_copy(dst, src)

                if t % 2 == 1:
                    j = t // 2
                    dw = nc.sync.dma_start(
                        out=zchunk[j].rearrange("(p k c) -> p k c", p=P, k=KT),
                        in_=znt[0][:, :, j * CHC : (j + 1) * CHC],
                    )
                    cc = nc.gpsimd.collective_compute(
                        "AllGather",
                        mybir.AluOpType.bypass,
                        ins=[zchunk[j].ap()],
                        outs=[gath_zc[j].ap()],
                        replica_groups=[list(range(N_CORES))],
                    )
                    _add_dep_helper(cc.ins, dw.ins, reason="ccz after chunk write")
                    ccz[j] = cc

            # ---- read remote blocks per chunk ----
            for j in range(NCH):
                for d in range(1, NBLK):
                    rv = (pid + d) % N_CORES
                    dz = nc.sync.dma_start(
                        out=znt[d][:, :, j * CHC : (j + 1) * CHC],
                        in_=gath_zc[j][rv].rearrange("(p k c) -> p k c", p=P, k=KT),
                    )
                    _add_dep_helper(dz.ins, ccz[j].ins, reason="read after ccz")

            # zero colsum accumulators
            for j in range(len(CS_NB)):
                nc.vector.memset(colT[j], 0.0)

            # ---- phase B: GEMM + exp row-sums + colsum accumulation ----
            for nb in range(NB):
                d_blk, half = nb // 2, nb % 2
                for mb in range(MB):
                    ps = psum_mm.tile([P, COLG], f32, tag="ps")
                    for kk in range(0, KT, 2):
                        nc.tensor.matmul(
                            ps,
                            lhsT=znt[0][:, kk : kk + 2, mb * P : (mb + 1) * P],
                            rhs=znt[d_blk][
                                :, kk : kk + 2, half * COLG : (half + 1) * COLG
                            ],
                            perf_mode=mybir.MatmulPerfMode.DoubleRow,
                            start=(kk == 0),
                            stop=(kk == KT - 2),
                        )
                    if nb == mb // 4:
                        off = (mb % 4) * P
                        nc.vector.copy_predicated(
                            out=ps[:, off : off + P], mask=ident_u8, data=negtile
                        )
                    if nb == 8 + mb // 4:
                        off = (mb % 4) * P
                        pos_scr = work.tile([P, P], f32, tag="pos_scr")
                        nc.vector.tensor_mul(pos_scr, ps[:, off : off + P], ident_f32)
                        nc.vector.tensor_reduce(
                            posq[:, mb : mb + 1],
                            pos_scr,
                            axis=mybir.AxisListType.X,
                            op=ALU.add,
                        )
                    ex = work.tile([P, COLG], bf16, tag="ex")
                    nc.scalar.activation(
                        ex, ps, AF.Exp, scale=INV_T / (SCALE * SCALE),
                        accum_out=accs[:, mb, nb : nb + 1],
                    )
                    if nb in CS_NB:
                        j = nb - 2
                        nc.vector.tensor_add(colT[j], colT[j], ex)

            # ---- colsum partition-reduce via ones-matmul ----
            for j in range(len(CS_NB)):
                cps = psum_cs.tile([P, 4], f32, tag="cps")
                for q in range(COLG // P):
                    nc.tensor.matmul(
                        cps[:, q : q + 1],
                        lhsT=colT[j][:, q * P : (q + 1) * P],
                        rhs=ones_col,
                        start=True,
                        stop=True,
                    )
                nc.scalar.copy(cs[:, 4 * j : 4 * j + 4], cps)

            # ---- pack (p-major) + AllGather ----
            rowsum = accp.tile([P, MB], f32, tag="rowsum")
            nc.vector.tensor_reduce(
                rowsum, accs, axis=mybir.AxisListType.X, op=ALU.add
            )
            pktv = pkt_dram.rearrange("(p r) -> p r", p=P)
            d1 = nc.sync.dma_start(out=pktv[:, 0:MB], in_=rowsum)
            d2 = nc.sync.dma_start(out=pktv[:, MB:PKTW], in_=cs)
            cc = nc.gpsimd.collective_compute(
                "AllGather",
                mybir.AluOpType.bypass,
                ins=[pkt_dram.ap()],
                outs=[gathered.ap()],
                replica_groups=[list(range(N_CORES))],
            )
            _add_dep_helper(cc.ins, d1.ins, reason="cc after pkt rowsum")
            _add_dep_helper(cc.ins, d2.ins, reason="cc after pkt cs")

            # ---- reassemble: one DMA for all ranks' packets ----
            G = accp.tile([P, N_CORES, PKTW], f32, tag="G")
            dg = nc.sync.dma_start(
                out=G, in_=gathered.rearrange("b (p r) -> p b r", p=P)
            )
            _add_dep_helper(dg.ins, cc.ins, reason="read gathered after cc")

            # tot[p,b,m] = G[p,b,m] + sum_d G[p,(b-d)%8, 8+(d-1)*8+m]
            tot = accp.tile([P, N_CORES, MB], f32, tag="tot")
            nc.vector.tensor_copy(tot, G[:, :, 0:MB])
            for d in (1, 2, 3):
                c0 = MB + (d - 1) * MB
                nc.vector.tensor_add(
                    tot[:, d:N_CORES, :],
                    tot[:, d:N_CORES, :],
                    G[:, 0 : N_CORES - d, c0 : c0 + MB],
                )
                nc.vector.tensor_add(
                    tot[:, 0:d, :],
                    tot[:, 0:d, :],
                    G[:, N_CORES - d : N_CORES, c0 : c0 + MB],
                )

            lnt = accp.tile([P, N_CORES, MB], f32, tag="lnt")
            nc.scalar.activation(lnt, tot, AF.Ln)
            gsum = accp.tile([P, 1], f32, tag="gsum")
            nc.vector.tensor_reduce(
                gsum, lnt, axis=mybir.AxisListType.XY, op=ALU.add
            )
            poss = accp.tile([P, MB], f32, tag="poss")
            nc.vector.tensor_scalar_mul(poss, posq, -INV_T / (SCALE * SCALE))
            psum_part = accp.tile([P, 1], f32, tag="psum_part")
            nc.vector.tensor_reduce(
                psum_part, poss, axis=mybir.AxisListType.X, op=ALU.add
            )
            part = accp.tile([P, 1], f32, tag="part")
            nc.vector.tensor_scalar_mul(part, gsum, 1.0 / N_CORES)
            nc.vector.tensor_add(part, part, psum_part)
            nc.sync.dma_start(out=out_dram[:, :], in_=part)

    nc.finalize()
    return nc


def _get_nc():
    if "nc" not in _CACHE:
        _CACHE["nc"] = _build_nc()
    return _CACHE["nc"]


def _run(z, trace=False):
    from concourse.bass_utils import run_bass_kernel_spmd

    z = np.ascontiguousarray(np.asarray(z, dtype=np.float32))
    assert z.shape == (N, D), z.shape
    nc = _get_nc()
    in_maps = [
        {"z": np.ascontiguousarray(np.roll(z, -ROWS_PER_CORE * c, axis=0))}
        for c in range(N_CORES)
    ]
    res = run_bass_kernel_spmd(
        nc, in_maps, core_ids=list(range(N_CORES)), trace=False
    )
    total = np.float64(0.0)
    for r in res.results:
        total += r["out"].astype(np.float64).sum()
    loss = np.float32(total / N)
    return loss, res


def kernel(z):
    loss, _ = _run(z, trace=False)
    return np.array(loss, dtype=np.float32)
